# revision 5
# baseline (speedup 1.0000x reference)
"""Trainium2 Bass kernel for windowed/sparse attention (nn_Attention_21732534518476).

Strategy (v2 - ACT-saturation design):
  - 8 NeuronCores, one attention head per core (HEADS == 8).
  - Host-side prep: transpose x -> xt, per-head projection weights (q/k
    replicated at two partition bases for PE row-tiling), gather+exp the
    relative-position bias table into a per-head dense [j, i] bf16 matrix,
    augment w_out with a b_out/8 row that rides on the softmax-sum row.
  - The Scalar engine's exp is the hard floor: B*N*N = 16.7M elements at
    (1024+172)/1.2 ns per 1024-wide call = 127.6 us.  Everything else is
    engineered to hide under that stream:
      * one linearized (batch, j-chunk) loop; per step: 4 score matmuls
        (row-tiled pairs) -> 2 exp calls -> 2 bias-mults (DVE) -> 4 attn@v
        matmuls (col-tiled pairs) accumulating in PSUM, with attn@v lagging
        2 steps so batch boundaries never stall the in-order PE stream.
      * qkv projections for batches 1-3 interleaved into the attention
        stream; v transposes for batch b+1 built near the end of batch b.
      * output projection (tail) of batch b interleaved into batch b+1.
      * softmax normalization moved to the host: the per-query divide by
        the softmax sum commutes with the output projection, so the device
        ships unnormalized projected outputs plus the sums row.
  - PSUM: 2x[128,1024] score tiles (4 banks) + [97,1024] attn@v accumulator
    (2 banks) + a 2-slot 1-bank ring for proj/transpose/out-proj tiles.
  - Host sums the 8 partial outputs (head reduction) after dividing each
    core's partial by its head's softmax sums.
"""

import os
import sys

sys.path.insert(0, "/opt/trn_rl_repo")
os.environ.setdefault("MYCRO_LOCAL_CACHE", "1")

import numpy as np
import ml_dtypes

BF = ml_dtypes.bfloat16

B, N, C = 4, 2048, 256
HEADS, D = 8, 32
BN = B * N  # 8192
JT = 16  # j chunks of 128 per batch
IB = 16  # i blocks of 512 over the full 8192
SCALE = D ** -0.5

_CACHE = {}

# Interleave schedule: global step g = b*16 + jc -> projection ib to emit.
PROJ_SCHED = {2: 4, 5: 5, 8: 6, 11: 7,
              20: 8, 22: 9, 24: 10, 26: 11,
              32: 12, 34: 13, 36: 14, 38: 15}
V1_SCHED = {14: 1, 30: 2, 46: 3}


def _build():
    from concourse import bass, mybir, bacc
    import concourse.tile as tile
    from concourse.masks import make_identity

    f32 = mybir.dt.float32
    bfl = mybir.dt.bfloat16
    Exp = mybir.ActivationFunctionType.Exp
    Copy = mybir.ActivationFunctionType.Copy
    mult = mybir.AluOpType.mult

    nc = bacc.Bacc(
        "TRN2",
        target_bir_lowering=False,
        debug=False,
        num_devices=8,
    )

    xt_ext = nc.dram_tensor("xt", [128, 2, BN], bfl, kind="ExternalInput")
    # projection weights, lhsT layout [c(128), cc, m]: qv cols [q,q,vT], k cols [k,k]
    wqv_ext = nc.dram_tensor("wqv", [128, 2, 96], bfl, kind="ExternalInput")
    wk_ext = nc.dram_tensor("wk", [128, 2, 64], bfl, kind="ExternalInput")
    ebias_ext = nc.dram_tensor("ebias", [128, JT, N], bfl, kind="ExternalInput")
    wout_ext = nc.dram_tensor("wout", [33, 256], bfl, kind="ExternalInput")
    out_ext = nc.dram_tensor("out", [128, 2, BN], bfl, kind="ExternalOutput")
    # unnormalized softmax sums: row 0 = i-half0, row 64 = i-half1 (per batch)
    sums_ext = nc.dram_tensor("sums", [65, B, 1024], bfl, kind="ExternalOutput")

    with tile.TileContext(nc) as tc:
        with (
            tc.tile_pool(name="const", bufs=1) as constp,
            tc.tile_pool(name="big", bufs=1) as bigp,
            tc.tile_pool(name="prp", bufs=8) as prp,
            tc.tile_pool(name="ptp", bufs=6) as ptp,
            tc.tile_pool(name="osbp", bufs=2) as osbp,
            tc.tile_pool(name="outp", bufs=2) as outp,
            tc.tile_pool(name="pst", bufs=2, space="PSUM") as pst,
            tc.tile_pool(name="oaccp", bufs=1, space="PSUM") as oaccp,
            tc.tile_pool(name="ring", bufs=2, space="PSUM") as ring,
        ):
            # warm the exp spline table during the initial DMAs
            warm = constp.tile([1, 8], f32, tag="warm")
            nc.gpsimd.memset(warm[:], 0.0)
            nc.scalar.activation(warm[:], warm[:], Exp)
            ident = constp.tile([128, 128], bfl, tag="ident")
            make_identity(nc, ident[:])
            wqv_sb = constp.tile([128, 2, 96], bfl, tag="wqv")
            nc.sync.dma_start(out=wqv_sb[:], in_=wqv_ext[:])
            wk_sb = constp.tile([128, 2, 64], bfl, tag="wk")
            nc.sync.dma_start(out=wk_sb[:], in_=wk_ext[:])
            wout_sb = constp.tile([97, 256], bfl, tag="wout")
            nc.sync.dma_start(out=wout_sb[0:33, :], in_=wout_ext[:])
            nc.sync.dma_start(out=wout_sb[64:97, :], in_=wout_ext[:])

            ebias_sb = bigp.tile([128, JT, N], bfl, tag="ebias")
            xt_sb = bigp.tile([128, 2, BN], bfl, tag="xt")
            q_sb = bigp.tile([96, IB, 512], bfl, tag="q")  # rows: q@0, q@32, vT@64
            k_sb = bigp.tile([64, IB, 512], bfl, tag="k")  # rows: k@0, k@32
            v1_sb = bigp.tile([128, B, JT, 33], bfl, tag="v1")
            nc.gpsimd.memset(v1_sb[:, :, :, 32:33], 1.0)

            # input DMAs: xt chunks (2 ib each) and ebias chunks interleaved so
            # early projections and early bias-mults are fed first
            def xt_dma(g2):
                nc.sync.dma_start(
                    out=xt_sb[:, :, g2 * 1024 : (g2 + 1) * 1024],
                    in_=xt_ext[:, :, g2 * 1024 : (g2 + 1) * 1024],
                )

            xt_dma(0)
            xt_dma(1)
            nc.sync.dma_start(out=ebias_sb[:, 0, :], in_=ebias_ext[:, 0, :])
            for g2 in range(2, 8):
                xt_dma(g2)
                nc.sync.dma_start(
                    out=ebias_sb[:, g2 - 1, :], in_=ebias_ext[:, g2 - 1, :]
                )
            for jc in range(7, JT):
                nc.sync.dma_start(out=ebias_sb[:, jc, :], in_=ebias_ext[:, jc, :])

            def proj(ib):
                psq = ring.tile([96, 512], f32, tag="r")
                for cc in range(2):
                    nc.tensor.matmul(
                        psq[:],
                        lhsT=wqv_sb[:, cc, :],
                        rhs=xt_sb[:, cc, ib * 512 : (ib + 1) * 512],
                        start=(cc == 0),
                        stop=(cc == 1),
                    )
                psk = ring.tile([64, 512], f32, tag="r")
                for cc in range(2):
                    nc.tensor.matmul(
                        psk[:],
                        lhsT=wk_sb[:, cc, :],
                        rhs=xt_sb[:, cc, ib * 512 : (ib + 1) * 512],
                        start=(cc == 0),
                        stop=(cc == 1),
                    )
                nc.vector.tensor_copy(q_sb[:, ib, :], psq[:])
                nc.vector.tensor_copy(k_sb[:, ib, :], psk[:])

            def v1_build(bq):
                tp = ring.tile([128, JT, 32], bfl, tag="r")
                for jh in range(JT):
                    j0 = bq * N + jh * 128
                    ib = j0 // 512
                    off = j0 % 512
                    nc.tensor.transpose(
                        tp[:, jh, :],
                        q_sb[64:96, ib, off : off + 128],
                        ident[64:96, 64:96],
                    )
                nc.vector.tensor_copy(v1_sb[:, bq, :, 0:32], tp[:])

            for ib in range(4):
                proj(ib)
            v1_build(0)

            # per-batch state built lazily inside the linearized loop
            o_acc = [None] * B
            o_sb = [None] * B
            out_t = [None] * B
            pts = {}

            def scores_exp_mult(g):
                b, jc = divmod(g, JT)
                j0 = b * N + jc * 128
                jb = j0 // 512
                off = j0 % 512
                for h in range(2):
                    st = pst.tile([128, 1024], f32, tag="st")
                    for t in range(2):
                        nc.tensor.matmul(
                            st[:, t * 512 : (t + 1) * 512],
                            lhsT=k_sb[32 * t : 32 * t + 32, jb, off : off + 128],
                            rhs=q_sb[32 * t : 32 * t + 32, 4 * b + 2 * h + t, :],
                            start=True,
                            stop=True,
                        )
                    pr = prp.tile([128, 1024], bfl, tag="pr")
                    nc.scalar.activation(pr[:], st[:], Exp)
                    pt = ptp.tile([128, 1024], bfl, tag="pt")
                    nc.vector.tensor_tensor(
                        pt[:],
                        pr[:],
                        ebias_sb[:, jc, h * 1024 : (h + 1) * 1024],
                        mult,
                    )
                    pts[(g, h)] = pt

            def attnv(g):
                b, jc = divmod(g, JT)
                if jc == 0:
                    o_acc[b] = oaccp.tile(
                        [97, 1024], f32, tag="oacc", name=f"oacc{b}"
                    )
                for h in range(2):
                    pt = pts.pop((g, h))
                    for s in range(2):
                        nc.tensor.matmul(
                            o_acc[b][64 * h : 64 * h + 33, s * 512 : (s + 1) * 512],
                            lhsT=v1_sb[:, b, jc, :],
                            rhs=pt[:, s * 512 : (s + 1) * 512],
                            start=(jc == 0),
                            stop=(jc == JT - 1),
                            skip_group_check=(h == 1),
                        )

            def tail_start(b, last=False):
                # unnormalized O^T (+ sums rows at partitions 32/96) -> SBUF
                o_sb[b] = osbp.tile([97, 1024], bfl, tag="osb", name=f"osb{b}")
                if last:
                    nc.scalar.activation(o_sb[b][:], o_acc[b][:], Copy)
                else:
                    nc.vector.tensor_copy(o_sb[b][:], o_acc[b][:])
                nc.sync.dma_start(out=sums_ext[:, b, :], in_=o_sb[b][32:97, :])
                out_t[b] = outp.tile(
                    [128, 2, 2048], bfl, tag="out_t", name=f"out_t{b}"
                )

            def tail_part(b, part, last=False):
                # part in 0..3 -> (s, cc); row-paired out-proj matmuls h=0/1
                s, cc = divmod(part, 2)
                ops = []
                for h in range(2):
                    op = ring.tile([128, 512], f32, tag="r")
                    nc.tensor.matmul(
                        op[:],
                        lhsT=wout_sb[64 * h : 64 * h + 33, cc * 128 : (cc + 1) * 128],
                        rhs=o_sb[b][64 * h : 64 * h + 33, s * 512 : (s + 1) * 512],
                        start=True,
                        stop=True,
                    )
                    ops.append(op)
                for h in range(2):
                    dst = out_t[b][:, cc, (2 * h + s) * 512 : (2 * h + s + 1) * 512]
                    if last and h == 1:
                        nc.scalar.activation(dst, ops[h][:], Copy)
                    else:
                        nc.vector.tensor_copy(dst, ops[h][:])

            def tail_flush(b):
                for cc in range(2):
                    nc.sync.dma_start(
                        out=out_ext[:, cc, b * N : (b + 1) * N],
                        in_=out_t[b][:, cc, :],
                    )

            for g in range(B * JT):
                b, jc = divmod(g, JT)
                scores_exp_mult(g)
                if g >= 2:
                    attnv(g - 2)
                if g in PROJ_SCHED:
                    proj(PROJ_SCHED[g])
                if g in V1_SCHED:
                    v1_build(V1_SCHED[g])
                if b >= 1:
                    if jc == 1:
                        tail_start(b - 1)
                    elif 2 <= jc <= 5:
                        tail_part(b - 1, jc - 2)
                    elif jc == 6:
                        tail_flush(b - 1)
            attnv(B * JT - 2)
            attnv(B * JT - 1)
            tail_start(B - 1, last=True)
            for part in range(4):
                tail_part(B - 1, part, last=True)
            tail_flush(B - 1)
    nc.compile()
    return nc


def _prep_inputs(x, w_qkv, bias_table, w_out, b_out, rel_index):
    x = np.asarray(x, dtype=np.float32)
    w_qkv = np.asarray(w_qkv, dtype=np.float32)
    bias_table = np.asarray(bias_table, dtype=np.float32)
    w_out = np.asarray(w_out, dtype=np.float32)
    b_out = np.asarray(b_out, dtype=np.float32)
    rel_index = np.asarray(rel_index)

    xt = np.ascontiguousarray(
        x.reshape(BN, C).T.reshape(2, 128, BN).transpose(1, 0, 2)
    ).astype(BF)

    # rel transposed so the gather lands directly in [j, i] order
    relT = np.ascontiguousarray(rel_index.reshape(N, N).T).reshape(-1)

    in_maps = []
    for h in range(HEADS):
        wq = w_qkv[:, h * D : (h + 1) * D] * SCALE
        wk = w_qkv[:, C + h * D : C + (h + 1) * D]
        wv = w_qkv[:, 2 * C + h * D : 2 * C + (h + 1) * D]
        qv = np.concatenate([wq, wq, wv], axis=1)  # (256, 96)
        kk = np.concatenate([wk, wk], axis=1)  # (256, 64)
        wqv_h = np.ascontiguousarray(
            qv.reshape(2, 128, 96).transpose(1, 0, 2)
        ).astype(BF)
        wk_h = np.ascontiguousarray(
            kk.reshape(2, 128, 64).transpose(1, 0, 2)
        ).astype(BF)

        ebias = np.exp(bias_table[:, h][relT].reshape(N, N))  # exp(bias) [j, i]
        ebias_h = np.ascontiguousarray(
            ebias.reshape(JT, 128, N).transpose(1, 0, 2)
        ).astype(BF)

        wout_h = np.concatenate(
            [w_out[h * D : (h + 1) * D, :], (b_out / HEADS)[None, :]], axis=0
        ).astype(BF)  # (33, 256)

        in_maps.append(
            {
                "xt": xt,
                "wqv": wqv_h,
                "wk": wk_h,
                "ebias": ebias_h,
                "wout": np.ascontiguousarray(wout_h),
            }
        )
    return in_maps


def _run(in_maps, trace=False, **kwargs):
    from concourse.bass_utils import run_bass_kernel_spmd

    if "nc" not in _CACHE:
        _CACHE["nc"] = _build()
    nc = _CACHE["nc"]
    res = run_bass_kernel_spmd(
        nc, in_maps, core_ids=list(range(8)), trace=trace, **kwargs
    )
    return res


def kernel(x, w_qkv, bias_table, w_out, b_out, rel_index):
    in_maps = _prep_inputs(x, w_qkv, bias_table, w_out, b_out, rel_index)
    res = _run(in_maps, trace=False)
    acc = np.zeros((256, BN), dtype=np.float32)
    for c in range(8):
        o = np.asarray(res.results[c]["out"], dtype=np.float32)  # (128, 2, 8192)
        sums = np.asarray(res.results[c]["sums"], dtype=np.float32)  # (65, B, 1024)
        s = np.concatenate([sums[0], sums[64]], axis=1).reshape(BN)  # per-i sums
        acc += o.transpose(1, 0, 2).reshape(256, BN) / s[None, :]
    out = acc.T.reshape(B, N, C).astype(np.float32)
    return out


# revision 10
# speedup vs baseline: 1.3183x; 1.3183x over previous
"""Trainium2 Bass kernel for windowed/sparse attention (nn_Attention_21732534518476).

Strategy (v2 - ACT-saturation design):
  - 8 NeuronCores, one attention head per core (HEADS == 8).
  - Host-side prep: transpose x -> xt, per-head projection weights (q/k
    replicated at two partition bases for PE row-tiling), gather+exp the
    relative-position bias table into a per-head dense [j, i] bf16 matrix,
    augment w_out with a b_out/8 row that rides on the softmax-sum row.
  - The Scalar engine's exp is the hard floor: B*N*N = 16.7M elements at
    (1024+172)/1.2 ns per 1024-wide call = 127.6 us.  Everything else is
    engineered to hide under that stream:
      * one linearized (batch, j-chunk) loop; per step: 4 score matmuls
        (row-tiled pairs) -> 2 exp calls -> 2 bias-mults (DVE) -> 4 attn@v
        matmuls (col-tiled pairs) accumulating in PSUM, with attn@v lagging
        2 steps so batch boundaries never stall the in-order PE stream.
      * qkv projections for batches 1-3 interleaved into the attention
        stream; v transposes for batch b+1 built near the end of batch b.
      * output projection (tail) of batch b interleaved into batch b+1.
      * softmax normalization moved to the host: the per-query divide by
        the softmax sum commutes with the output projection, so the device
        ships unnormalized projected outputs plus the sums row.
  - PSUM: 2x[128,1024] score tiles (4 banks) + [97,1024] attn@v accumulator
    (2 banks) + a 2-slot 1-bank ring for proj/transpose/out-proj tiles.
  - Host sums the 8 partial outputs (head reduction) after dividing each
    core's partial by its head's softmax sums.
"""

import os
import sys

sys.path.insert(0, "/opt/trn_rl_repo")
os.environ.setdefault("MYCRO_LOCAL_CACHE", "1")

import numpy as np
import ml_dtypes

BF = ml_dtypes.bfloat16

B, N, C = 4, 2048, 256
HEADS, D = 8, 32
BN = B * N  # 8192
JT = 16  # j chunks of 128 per batch
IB = 16  # i blocks of 512 over the full 8192
SCALE = D ** -0.5

_CACHE = {}

# Interleave schedule: global step g = b*16 + jc -> projection ib to emit.
PROJ_SCHED = {2: 4, 5: 5, 8: 6, 11: 7,
              20: 8, 22: 9, 24: 10, 26: 11,
              32: 12, 34: 13, 36: 14, 38: 15}


def _build():
    from concourse import bass, mybir, bacc
    import concourse.tile as tile
    from concourse.masks import make_identity

    f32 = mybir.dt.float32
    bfl = mybir.dt.bfloat16
    Exp = mybir.ActivationFunctionType.Exp
    Copy = mybir.ActivationFunctionType.Copy
    mult = mybir.AluOpType.mult

    nc = bacc.Bacc(
        "TRN2",
        target_bir_lowering=False,
        debug=False,
        num_devices=8,
    )

    xt_ext = nc.dram_tensor("xt", [128, 2, BN], bfl, kind="ExternalInput")
    # projection weights, lhsT layout [c(128), cc, m]: qv cols [q,q,vT], k cols [k,k]
    wqv_ext = nc.dram_tensor("wqv", [128, 2, 96], bfl, kind="ExternalInput")
    wk_ext = nc.dram_tensor("wk", [128, 2, 64], bfl, kind="ExternalInput")
    ebias_ext = nc.dram_tensor("ebias", [128, JT, N], bfl, kind="ExternalInput")
    wout_ext = nc.dram_tensor("wout", [33, 256], bfl, kind="ExternalInput")
    out_ext = nc.dram_tensor("out", [128, 2, BN], bfl, kind="ExternalOutput")
    # unnormalized softmax sums: row 0 = i-half0, row 64 = i-half1 (per batch)
    sums_ext = nc.dram_tensor("sums", [65, B, 1024], bfl, kind="ExternalOutput")

    with tile.TileContext(nc) as tc:
        with (
            tc.tile_pool(name="const", bufs=1) as constp,
            tc.tile_pool(name="big", bufs=1) as bigp,
            tc.tile_pool(name="prp", bufs=12) as prp,
            tc.tile_pool(name="ptp", bufs=6) as ptp,
            tc.tile_pool(name="osbp", bufs=2) as osbp,
            tc.tile_pool(name="outp", bufs=2) as outp,
            tc.tile_pool(name="pst", bufs=2, space="PSUM") as pst,
            tc.tile_pool(name="oaccp", bufs=1, space="PSUM") as oaccp,
            tc.tile_pool(name="ring", bufs=2, space="PSUM") as ring,
        ):
            # warm the exp spline table during the initial DMAs
            warm = constp.tile([1, 8], f32, tag="warm")
            nc.gpsimd.memset(warm[:], 0.0)
            nc.scalar.activation(warm[:], warm[:], Exp)
            ident = constp.tile([128, 128], bfl, tag="ident")
            make_identity(nc, ident[:])

            ebias_sb = bigp.tile([128, JT, N], bfl, tag="ebias")
            xt_sb = bigp.tile([128, 2, BN], bfl, tag="xt")
            q_sb = bigp.tile([96, IB, 512], bfl, tag="q")  # rows: q@0, q@32, vT@64
            k_sb = bigp.tile([64, IB, 512], bfl, tag="k")  # rows: k@0, k@32
            v1_sb = bigp.tile([128, B, JT, 33], bfl, tag="v1")
            nc.gpsimd.memset(v1_sb[:, :, :, 32:33], 1.0)

            # input DMAs: xt chunks (2 ib each) first (they gate the first
            # projections/scores), then ebias chunks interleaved
            def xt_dma(g2):
                nc.sync.dma_start(
                    out=xt_sb[:, :, g2 * 1024 : (g2 + 1) * 1024],
                    in_=xt_ext[:, :, g2 * 1024 : (g2 + 1) * 1024],
                )

            wqv_sb = constp.tile([128, 2, 96], bfl, tag="wqv")
            nc.sync.dma_start(out=wqv_sb[:], in_=wqv_ext[:])
            wk_sb = constp.tile([128, 2, 64], bfl, tag="wk")
            nc.sync.dma_start(out=wk_sb[:], in_=wk_ext[:])
            xt_dma(0)
            xt_dma(1)
            nc.sync.dma_start(out=ebias_sb[:, 0, :], in_=ebias_ext[:, 0, :])
            nc.sync.dma_start(out=ebias_sb[:, 1, :], in_=ebias_ext[:, 1, :])
            wout_sb = constp.tile([97, 256], bfl, tag="wout")
            nc.sync.dma_start(out=wout_sb[0:33, :], in_=wout_ext[:])
            nc.sync.dma_start(out=wout_sb[64:97, :], in_=wout_ext[:])
            for g2 in range(2, 8):
                xt_dma(g2)
                nc.sync.dma_start(out=ebias_sb[:, g2, :], in_=ebias_ext[:, g2, :])
            for jc in range(8, JT):
                nc.sync.dma_start(out=ebias_sb[:, jc, :], in_=ebias_ext[:, jc, :])

            # warm the PE's HAM clock gate during the DMA wait so projections
            # and first scores run at full clock
            wps = ring.tile([128, 128], f32, tag="r")
            for _ in range(26):
                nc.tensor.matmul(
                    wps[:], lhsT=ident[:], rhs=ident[:, 0:128],
                    start=True, stop=True,
                )

            def proj_q(ib):
                psq = ring.tile([96, 512], f32, tag="r")
                for cc in range(2):
                    nc.tensor.matmul(
                        psq[:],
                        lhsT=wqv_sb[:, cc, :],
                        rhs=xt_sb[:, cc, ib * 512 : (ib + 1) * 512],
                        start=(cc == 0),
                        stop=(cc == 1),
                    )
                nc.vector.tensor_copy(q_sb[:, ib, :], psq[:])

            def proj_k(ib):
                psk = ring.tile([64, 512], f32, tag="r")
                for cc in range(2):
                    nc.tensor.matmul(
                        psk[:],
                        lhsT=wk_sb[:, cc, :],
                        rhs=xt_sb[:, cc, ib * 512 : (ib + 1) * 512],
                        start=(cc == 0),
                        stop=(cc == 1),
                    )
                nc.vector.tensor_copy(k_sb[:, ib, :], psk[:])

            def proj(ib):
                proj_q(ib)
                proj_k(ib)

            def v1_chunk(bq, quarter):
                # 4 transposes into the persistent per-batch v1 staging tile
                tp = ring.tile([128, 4, 32], bfl, tag="r", name=f"tp{bq}_{quarter}")
                for t in range(4):
                    jh = quarter * 4 + t
                    j0 = bq * N + jh * 128
                    ib = j0 // 512
                    off = j0 % 512
                    nc.tensor.transpose(
                        tp[:, t, :],
                        q_sb[64:96, ib, off : off + 128],
                        ident[64:96, 64:96],
                    )
                nc.vector.tensor_copy(
                    v1_sb[:, bq, quarter * 4 : quarter * 4 + 4, 0:32], tp[:]
                )

            def v1_build(bq):
                for quarter in range(4):
                    v1_chunk(bq, quarter)

            for ib in range(4):
                proj(ib)
            v1_build(0)

            # per-batch state built lazily inside the linearized loop
            o_acc = [None] * B
            o_sb = [None] * B
            out_t = [None] * B
            pts = {}

            def scores_exp_mult(g):
                b, jc = divmod(g, JT)
                j0 = b * N + jc * 128
                jb = j0 // 512
                off = j0 % 512
                for h in range(2):
                    st = pst.tile([128, 1024], f32, tag="st")
                    for t in range(2):
                        nc.tensor.matmul(
                            st[:, t * 512 : (t + 1) * 512],
                            lhsT=k_sb[32 * t : 32 * t + 32, jb, off : off + 128],
                            rhs=q_sb[32 * t : 32 * t + 32, 4 * b + 2 * h + t, :],
                            start=True,
                            stop=True,
                        )
                    pr = prp.tile([128, 1024], bfl, tag="pr")
                    nc.scalar.activation(pr[:], st[:], Exp)
                    pt = ptp.tile([128, 1024], bfl, tag="pt")
                    nc.vector.tensor_tensor(
                        pt[:],
                        pr[:],
                        ebias_sb[:, jc, h * 1024 : (h + 1) * 1024],
                        mult,
                    )
                    pts[(g, h)] = pt

            def attnv(g):
                b, jc = divmod(g, JT)
                if jc == 0:
                    o_acc[b] = oaccp.tile(
                        [97, 1024], f32, tag="oacc", name=f"oacc{b}"
                    )
                for h in range(2):
                    pt = pts.pop((g, h))
                    for s in range(2):
                        nc.tensor.matmul(
                            o_acc[b][64 * h : 64 * h + 33, s * 512 : (s + 1) * 512],
                            lhsT=v1_sb[:, b, jc, :],
                            rhs=pt[:, s * 512 : (s + 1) * 512],
                            start=(jc == 0),
                            stop=(jc == JT - 1),
                            skip_group_check=(h == 1),
                        )

            def tail_start(b, last=False):
                # unnormalized O^T (+ sums rows at partitions 32/96) -> SBUF
                o_sb[b] = osbp.tile([97, 1024], bfl, tag="osb", name=f"osb{b}")
                if last:
                    nc.scalar.activation(o_sb[b][:], o_acc[b][:], Copy)
                else:
                    nc.vector.tensor_copy(o_sb[b][:], o_acc[b][:])
                nc.sync.dma_start(out=sums_ext[:, b, :], in_=o_sb[b][32:97, :])
                out_t[b] = outp.tile(
                    [128, 2, 2048], bfl, tag="out_t", name=f"out_t{b}"
                )

            def tail_part(b, part, last=False):
                # part in 0..3 -> (s, cc); row-paired out-proj matmuls h=0/1
                s, cc = divmod(part, 2)
                ops = []
                for h in range(2):
                    op = ring.tile([128, 512], f32, tag="r")
                    nc.tensor.matmul(
                        op[:],
                        lhsT=wout_sb[64 * h : 64 * h + 33, cc * 128 : (cc + 1) * 128],
                        rhs=o_sb[b][64 * h : 64 * h + 33, s * 512 : (s + 1) * 512],
                        start=True,
                        stop=True,
                    )
                    ops.append(op)
                for h in range(2):
                    dst = out_t[b][:, cc, (2 * h + s) * 512 : (2 * h + s + 1) * 512]
                    if last and h == 1:
                        nc.scalar.activation(dst, ops[h][:], Copy)
                    else:
                        nc.vector.tensor_copy(dst, ops[h][:])

            def tail_flush(b):
                for cc in range(2):
                    nc.sync.dma_start(
                        out=out_ext[:, cc, b * N : (b + 1) * N],
                        in_=out_t[b][:, cc, :],
                    )

            for g in range(B * JT):
                b, jc = divmod(g, JT)
                scores_exp_mult(g)
                if g >= 2:
                    attnv(g - 2)
                if g in PROJ_SCHED:
                    proj_q(PROJ_SCHED[g])
                if g - 1 in PROJ_SCHED:
                    proj_k(PROJ_SCHED[g - 1])
                if jc in (11, 12, 13, 14) and b < B - 1:
                    v1_chunk(b + 1, jc - 11)
                if b >= 1:
                    if jc == 1:
                        tail_start(b - 1)
                    elif 2 <= jc <= 5:
                        tail_part(b - 1, jc - 2)
                    elif jc == 6:
                        tail_flush(b - 1)
            attnv(B * JT - 2)
            attnv(B * JT - 1)
            tail_start(B - 1, last=True)
            for part in range(4):
                tail_part(B - 1, part, last=True)
            tail_flush(B - 1)
    nc.compile()
    return nc


def _prep_inputs(x, w_qkv, bias_table, w_out, b_out, rel_index):
    x = np.asarray(x, dtype=np.float32)
    w_qkv = np.asarray(w_qkv, dtype=np.float32)
    bias_table = np.asarray(bias_table, dtype=np.float32)
    w_out = np.asarray(w_out, dtype=np.float32)
    b_out = np.asarray(b_out, dtype=np.float32)
    rel_index = np.asarray(rel_index)

    xt = np.ascontiguousarray(
        x.reshape(BN, C).T.reshape(2, 128, BN).transpose(1, 0, 2)
    ).astype(BF)

    # rel transposed so the gather lands directly in [j, i] order
    relT = np.ascontiguousarray(rel_index.reshape(N, N).T).reshape(-1)

    in_maps = []
    for h in range(HEADS):
        wq = w_qkv[:, h * D : (h + 1) * D] * SCALE
        wk = w_qkv[:, C + h * D : C + (h + 1) * D]
        wv = w_qkv[:, 2 * C + h * D : 2 * C + (h + 1) * D]
        qv = np.concatenate([wq, wq, wv], axis=1)  # (256, 96)
        kk = np.concatenate([wk, wk], axis=1)  # (256, 64)
        wqv_h = np.ascontiguousarray(
            qv.reshape(2, 128, 96).transpose(1, 0, 2)
        ).astype(BF)
        wk_h = np.ascontiguousarray(
            kk.reshape(2, 128, 64).transpose(1, 0, 2)
        ).astype(BF)

        ebias = np.exp(bias_table[:, h][relT].reshape(N, N))  # exp(bias) [j, i]
        ebias_h = np.ascontiguousarray(
            ebias.reshape(JT, 128, N).transpose(1, 0, 2)
        ).astype(BF)

        wout_h = np.concatenate(
            [w_out[h * D : (h + 1) * D, :], (b_out / HEADS)[None, :]], axis=0
        ).astype(BF)  # (33, 256)

        in_maps.append(
            {
                "xt": xt,
                "wqv": wqv_h,
                "wk": wk_h,
                "ebias": ebias_h,
                "wout": np.ascontiguousarray(wout_h),
            }
        )
    return in_maps


def _run(in_maps, trace=False, **kwargs):
    from concourse.bass_utils import run_bass_kernel_spmd

    if "nc" not in _CACHE:
        _CACHE["nc"] = _build()
    nc = _CACHE["nc"]
    res = run_bass_kernel_spmd(
        nc, in_maps, core_ids=list(range(8)), trace=trace, **kwargs
    )
    return res


def kernel(x, w_qkv, bias_table, w_out, b_out, rel_index):
    in_maps = _prep_inputs(x, w_qkv, bias_table, w_out, b_out, rel_index)
    res = _run(in_maps, trace=False)
    acc = np.zeros((256, BN), dtype=np.float32)
    for c in range(8):
        o = np.asarray(res.results[c]["out"], dtype=np.float32)  # (128, 2, 8192)
        sums = np.asarray(res.results[c]["sums"], dtype=np.float32)  # (65, B, 1024)
        s = np.concatenate([sums[0], sums[64]], axis=1).reshape(BN)  # per-i sums
        acc += o.transpose(1, 0, 2).reshape(256, BN) / s[None, :]
    out = acc.T.reshape(B, N, C).astype(np.float32)
    return out
